# revision 1
# baseline (speedup 1.0000x reference)
"""Trainium2 Bass kernel for nn_DistanceProbe.

Computes, for batch [B=8, S=2048, H=768] and proj [H=768, R=768]:
    t  = batch @ proj                      # [B, S, R]
    d2 = relu(||t_i||^2 + ||t_j||^2 - 2 t_i . t_j)   # [B, S, S]

Sharding: data-parallel over B across the 8 NeuronCores (one batch
element per core). Each core receives its batch slice pre-transposed
(xT = batch[b].T, [H, S]) so the contraction dim H lands on SBUF
partitions without any on-device transpose.

Per-core device algorithm (all matmuls in float32r = full-rate fp32):
  1. tT[r, s]   = sum_h proj[h, r] * xT[h, s]        (PE, K=H)
  2. sq[s]      = sum_r tT[r, s]^2                   (DVE square + ones-matmul)
  3. psum[i, j] = sum_r tT[r, i] * tT[r, j]          (PE, K=R)
  4. out[i, j]  = relu(-2*psum + sq_j + sq_i)        (DVE stt + ACT relu w/ bias)

`reps` repeats the whole body inside one NEFF (used by test.py to
measure steady-state HW time by differencing two rep counts).
"""

import numpy as np

import concourse.bass as bass
import concourse.tile as tile
from concourse import bacc
from concourse import masks
from concourse import mybir
from concourse.bass_utils import run_bass_kernel_spmd

B, S, H, R = 8, 2048, 768, 768
N_CORES = 8
P = 128          # SBUF partitions
NC_ = 512        # matmul moving free dim (one PSUM bank of fp32)
HT = H // P      # 6  k-tiles over H
RT = R // P      # 6  k-tiles over R
IT = S // P      # 16 output row tiles
SC = S // NC_    # 4  512-wide column chunks

F32 = mybir.dt.float32


def build_nc(mm_dtype=mybir.dt.float32r, reps=1, symmetric=True):
    nc = bacc.Bacc("TRN2", target_bir_lowering=False, debug=False,
                   num_devices=N_CORES)

    xT_d = nc.dram_tensor("xT", [H, S], mm_dtype, kind="ExternalInput")
    pj_d = nc.dram_tensor("proj", [H, R], mm_dtype, kind="ExternalInput")
    out_d = nc.dram_tensor("out", [S, S], F32, kind="ExternalOutput")

    with tile.TileContext(nc) as tc:
        with tc.tile_pool(name="persist", bufs=1) as sb, \
             tc.tile_pool(name="stage", bufs=4) as stg, \
             tc.tile_pool(name="pmm", bufs=2, space="PSUM") as pmm, \
             tc.tile_pool(name="psq", bufs=1, space="PSUM") as psq, \
             tc.tile_pool(name="pd", bufs=3, space="PSUM") as pdp:

            xT_sb = [sb.tile([P, S], mm_dtype, name=f"xT{i}", tag=f"xT{i}")
                     for i in range(HT)]
            pj_sb = [sb.tile([P, R], mm_dtype, name=f"pj{i}", tag=f"pj{i}")
                     for i in range(HT)]
            tT_sb = [sb.tile([P, S], mm_dtype, name=f"tT{i}", tag=f"tT{i}")
                     for i in range(RT)]
            sqj = sb.tile([P, S], F32, name="sqj", tag="sqj")
            sqrow = sb.tile([1, S], mm_dtype, name="sqrow", tag="sqrow")
            sqrow_f = sb.tile([1, S], F32, name="sqrow_f", tag="sqrowf")
            sqcol = sb.tile([P, IT], F32, name="sqcol", tag="sqcol")
            ones_col = sb.tile([P, 1], mm_dtype, name="ones_col", tag="onc")
            ones_row = sb.tile([1, P], mm_dtype, name="ones_row", tag="onr")
            onesf_col = sb.tile([P, 1], F32, name="onesf_col", tag="onfc")
            onesf_row = sb.tile([1, P], F32, name="onesf_row", tag="onfr")

            nc.vector.memset(onesf_col[:], 1.0)
            nc.vector.memset(onesf_row[:], 1.0)
            nc.vector.tensor_copy(ones_col[:], onesf_col[:])
            nc.vector.tensor_copy(ones_row[:], onesf_row[:])
            if symmetric:
                ident = sb.tile([P, P], F32, name="ident", tag="ident")
                masks.make_identity(nc, ident[:])

            def emit_body():
                # loads: proj first (every matmul group needs all of it)
                for ht in range(HT):
                    nc.sync.dma_start(pj_sb[ht][:],
                                      pj_d[ht * P:(ht + 1) * P, :])
                for sc in range(SC):
                    for ht in range(HT):
                        nc.sync.dma_start(
                            xT_sb[ht][:, sc * NC_:(sc + 1) * NC_],
                            xT_d[ht * P:(ht + 1) * P, sc * NC_:(sc + 1) * NC_])

                # phase B: tT = projT @ x  (tT[r, s]); squares and the
                # sq row-reduction are interleaved per column chunk so the
                # DVE squares overlap the next chunk's PE matmuls
                for sc in range(SC):
                    sq_acc = stg.tile([P, NC_], mm_dtype, name="sq_acc",
                                      tag="sqacc", bufs=2)
                    for rt in range(RT):
                        pt = pmm.tile([P, NC_], F32, name="pt", tag="pt")
                        for ht in range(HT):
                            nc.tensor.matmul(
                                pt[:],
                                pj_sb[ht][:, rt * P:(rt + 1) * P],
                                xT_sb[ht][:, sc * NC_:(sc + 1) * NC_],
                                start=(ht == 0), stop=(ht == HT - 1))
                        nc.scalar.copy(tT_sb[rt][:, sc * NC_:(sc + 1) * NC_],
                                       pt[:])
                        tch = tT_sb[rt][:, sc * NC_:(sc + 1) * NC_]
                        if rt == 0:
                            nc.vector.tensor_mul(sq_acc[:], tch, tch)
                        else:
                            sq_t = stg.tile([P, NC_], mm_dtype, name="sq_t",
                                            tag="sqtmp", bufs=2)
                            nc.vector.tensor_mul(sq_t[:], tch, tch)
                            nc.vector.tensor_add(sq_acc[:], sq_acc[:],
                                                 sq_t[:])
                    sq_ps = psq.tile([1, NC_], F32, name="sq_ps", tag="sq")
                    nc.tensor.matmul(sq_ps[:], ones_col[:], sq_acc[:],
                                     start=True, stop=True)
                    nc.vector.tensor_copy(sqrow[0:1, sc * NC_:(sc + 1) * NC_],
                                          sq_ps[:])
                    nc.vector.tensor_copy(
                        sqrow_f[0:1, sc * NC_:(sc + 1) * NC_], sq_ps[:])

                # sq broadcast across partitions (ones_row^T @ sqrow)
                for sc in range(SC):
                    bc = pmm.tile([P, NC_], F32, name="bc", tag="pt")
                    nc.tensor.matmul(bc[:], ones_row[:],
                                     sqrow[0:1, sc * NC_:(sc + 1) * NC_],
                                     start=True, stop=True)
                    nc.vector.tensor_copy(sqj[:, sc * NC_:(sc + 1) * NC_],
                                          bc[:])

                # sq column form: 16x PE transpose of [1,128] slices
                for it in range(IT):
                    tp = pmm.tile([P, 1], F32, name="tp", tag="pt")
                    nc.tensor.transpose(tp[:],
                                        sqrow_f[0:1, it * P:(it + 1) * P],
                                        onesf_row[0:1, 0:1])
                    nc.vector.tensor_copy(sqcol[:, it:it + 1], tp[:])

                # phase D: dots + fused epilogue (jc-major so mirror
                # chunks batch 4 consecutive source rows)
                def emit_tile(it, jc, strip):
                    j0 = max(jc * NC_, it * P) if symmetric else jc * NC_
                    w = (jc + 1) * NC_ - j0
                    off = 0
                    if 0 < w < 256:
                        # sub-256 f32r matmuls run at 1/4 rate; widen
                        # leftward and discard the overlap columns
                        off = 256 - w
                        j0 -= off
                        w = 256
                    pd = pdp.tile([P, w], F32, name="pd", tag="pd")
                    for rt in range(RT):
                        nc.tensor.matmul(
                            pd[:],
                            tT_sb[rt][:, it * P:(it + 1) * P],
                            tT_sb[rt][:, j0:j0 + w],
                            start=(rt == 0), stop=(rt == RT - 1))
                    jv = j0 + off      # first valid output column
                    wv = w - off
                    st = stg.tile([P, wv], F32, name="st", tag="st", bufs=3)
                    nc.vector.scalar_tensor_tensor(
                        st[:], pd[:, off:w], -2.0,
                        sqj[:, jv:jv + wv],
                        mybir.AluOpType.mult, mybir.AluOpType.add)
                    st2 = stg.tile([P, wv], F32, name="st2", tag="st2",
                                   bufs=11)
                    nc.scalar.activation(
                        st2[:], st[:], mybir.ActivationFunctionType.Relu,
                        bias=sqcol[:, it:it + 1], scale=1.0)
                    nc.sync.dma_start(
                        out_d[it * P:(it + 1) * P, jv:jv + wv], st2[:])
                    strip[it] = (st2, jv)

                def flush_group(jc, it0, it1, strip):
                    # mirror blocks (it, jt) -> (jt, it) for it in
                    # [it0, it1], one [128, <=512] chunk per dest row jt
                    for jt in range(jc * (NC_ // P), (jc + 1) * (NC_ // P)):
                        its = [it for it in range(it0, it1 + 1) if it < jt]
                        if not its:
                            continue
                        cw = len(its) * P
                        mp = pmm.tile([P, cw], F32, name="mp", tag="mp",
                                      bufs=2)
                        for k, it in enumerate(its):
                            st2_t, jv_t = strip[it]
                            nc.tensor.transpose(
                                mp[:, k * P:(k + 1) * P],
                                st2_t[:, jt * P - jv_t:jt * P - jv_t + P],
                                ident[:])
                        mir = stg.tile([P, cw], F32, name="mir", tag="mir",
                                       bufs=6)
                        nc.scalar.copy(mir[:], mp[:])
                        nc.sync.dma_start(
                            out_d[jt * P:(jt + 1) * P,
                                  its[0] * P:(its[0] + len(its)) * P],
                            mir[:])

                if symmetric:
                    for jc in reversed(range(SC)):
                        maxit = jc * (NC_ // P) + (NC_ // P) - 1
                        strip = {}
                        groups = []
                        for it in range(0, maxit + 1):
                            emit_tile(it, jc, strip)
                            if it % 4 == 3 or it == maxit:
                                groups.append((it - it % 4, it))
                            # flush with one-group delay so PE never waits
                            # on this tile's DVE/ACT epilogue
                            if len(groups) > 1:
                                g = groups.pop(0)
                                flush_group(jc, g[0], g[1], strip)
                        for g in groups:
                            flush_group(jc, g[0], g[1], strip)
                else:
                    strip = {}
                    for it in range(IT):
                        for jc in range(SC):
                            emit_tile(it, jc, strip)

            for _ in range(reps):
                emit_body()

    nc.finalize()
    return nc


_NC_CACHE = {}


def get_nc(mm_dtype=mybir.dt.float32r, reps=1, symmetric=True):
    key = (str(mm_dtype), reps, symmetric)
    if key not in _NC_CACHE:
        _NC_CACHE[key] = build_nc(mm_dtype, reps, symmetric)
    return _NC_CACHE[key]


def make_in_maps(batch, proj):
    proj = np.ascontiguousarray(proj, dtype=np.float32)
    return [
        {"xT": np.ascontiguousarray(batch[b].T, dtype=np.float32),
         "proj": proj}
        for b in range(B)
    ]


def kernel(batch, proj):
    assert batch.shape == (B, S, H) and proj.shape == (H, R)
    nc = get_nc()
    in_maps = make_in_maps(batch, proj)
    res = run_bass_kernel_spmd(nc, in_maps, core_ids=list(range(N_CORES)))
    out = np.stack([res.results[b]["out"] for b in range(B)], axis=0)
    return out.astype(np.float32, copy=False)

